# revision 26
# baseline (speedup 1.0000x reference)
"""GQA (no RoPE) Trainium2 kernel, 8 NeuronCores — v4.

Sharding: 2 batches x 4 shards; each shard = 2 KV groups + 8 query heads,
processed as 4 pairsets (group0 head i at partitions 0:64, group1 head i
at 64:128); score matmuls for the two heads occupy disjoint PE row groups
and run concurrently.

v4 restructure vs v3 (trace-driven):
- All DRAM inputs repacked host-side to [128, N] wide-line layouts
  (weights went from 256B-1KB DMA lines to 2-16KB; far fewer descriptors).
- Attention pipelines INTO the input-DMA phase: qproj -> kproj ->
  scores+exp p0,p1 -> vproj (x) av_p0 -> AG0 -> scores+exp p2 -> av_p1 ->
  AG1 -> scores+exp p3 -> av_p2 -> AG2 -> wave0 -> av_p3 -> AG3 ->
  wave1..3.  (v3 ran all projections then attention: exp started ~83us.)
- Scores evacuate PSUM->SBUF via DVE with the causal mask folded in as a
  -100 additive mask on the diagonal block (pre-exp), so scalar runs exp
  in-place back-to-back and PSUM score tiles recycle at DVE speed.
- af gather-readback DMAs moved to the gpsimd queue right after each AG
  (prefetch; the sync queue never waits on a collective).
- PE warmup chain on tri while the first weights stream in (HAM warm
  before qproj).
- osum accumulates in bf16 (2x DVE).

Self-contained: hardcodes B=2, S=1024, D=2048, G=8, HG=4, HD=64.
"""

import os
import sys

sys.path.insert(0, "/opt/trn_rl_repo")

import numpy as np
import ml_dtypes

import concourse.bass as bass
import concourse.mybir as mybir
import concourse.tile as tile
from concourse import bacc
from concourse import bass_utils

BF16 = mybir.dt.bfloat16
F32 = mybir.dt.float32
AF = mybir.ActivationFunctionType

B, S, D = 2, 1024, 2048
G, HG, HD = 8, 4, 64
P = 128
NCORES = 8
GPC = 2
NPS = 4                          # pairsets per core
CQ = GPC * HG * HD               # 512
CK = GPC * HD                    # 128
CO = D // 4                      # 512
DC = D // P                      # 16
SC = S // P                      # 8
SEG = 512
NWARM = 72


def _build_nc():
    nc = bacc.Bacc(
        "TRN2",
        target_bir_lowering=False,
        debug=False,
        enable_asserts=False,
        num_devices=NCORES,
    )

    qtp = nc.dram_tensor("qtp", [P, DC * S], BF16, kind="ExternalInput").ap()
    ktp = nc.dram_tensor("ktp", [P, DC * S], BF16, kind="ExternalInput").ap()
    vtp = nc.dram_tensor("vtp", [P, DC * S], BF16, kind="ExternalInput").ap()
    wqp = nc.dram_tensor("wqp", [P, DC * CQ], BF16, kind="ExternalInput").ap()
    wkp = nc.dram_tensor("wkp", [P, DC * CK], BF16, kind="ExternalInput").ap()
    wvp = nc.dram_tensor("wvp", [P, DC * CK], BF16, kind="ExternalInput").ap()
    wop = nc.dram_tensor("wop", [P, DC * CO], BF16, kind="ExternalInput").ap()
    bo = nc.dram_tensor("bo", [1, CO], BF16, kind="ExternalInput").ap()
    tri = nc.dram_tensor("tri", [P, P], BF16, kind="ExternalInput").ap()
    mneg = nc.dram_tensor("mneg", [P, P], F32, kind="ExternalInput").ap()
    out = nc.dram_tensor("out", [S, CO], F32, kind="ExternalOutput").ap()

    with tile.TileContext(nc) as tc:
        with (
            tc.tile_pool(name="consts", bufs=1) as cp,
            tc.tile_pool(name="res", bufs=1) as rp,
            tc.tile_pool(name="wts", bufs=1) as wp,
            tc.tile_pool(name="xq", bufs=8) as xp,
            tc.tile_pool(name="psS", bufs=4, space="PSUM") as psS,
            tc.tile_pool(name="psO", bufs=4, space="PSUM") as psO,
            tc.tile_pool(name="dram", bufs=1, space="DRAM") as dp,
            tc.tile_pool(name="sx", bufs=48) as sxp,
            tc.tile_pool(name="nrm", bufs=2) as npool,
            tc.tile_pool(name="af", bufs=4) as afp,
            tc.tile_pool(name="osb", bufs=2) as op,
        ):
            tri_sb = cp.tile([P, P], BF16)
            nc.sync.dma_start(tri_sb[:], tri[:])
            mneg_sb = cp.tile([P, P], F32)
            nc.sync.dma_start(mneg_sb[:], mneg[:])
            bo_sb = cp.tile([1, CO], BF16)
            nc.sync.dma_start(bo_sb[:], bo[:])
            ones_sb = cp.tile([1, P], BF16)
            nc.vector.memset(ones_sb[:], 1.0)
            ones64f = cp.tile([1, HD], F32)
            nc.vector.memset(ones64f[:], 1.0)

            kt2 = rp.tile([P, S], BF16)
            qt2 = rp.tile([P, NPS, S], BF16)
            vaug = rp.tile([P, SC, GPC, HD + 1], BF16)
            attn_sb = rp.tile([P, NPS, S], BF16)
            osum = [rp.tile([P, CO], BF16, name=f"osum{s}") for s in range(SC)]
            nc.vector.memset(vaug[:, :, :, HD:HD + 1], 1.0)

            wq_sb = wp.tile([P, DC * CQ], BF16)
            wk_sb = wp.tile([P, DC * CK], BF16)
            wv_sb = wp.tile([P, DC * CK], BF16)
            wo_sb = wp.tile([P, DC * CO], BF16)

            # activation chunks share one 8-slot rotation; ALLOCATION order is
            # k, q, v so the v tiles land in k's slots (kproj is done by the
            # time V streams in), while DMA trigger order stays q-first
            k4 = [xp.tile([P, 4 * S], BF16, tag="xc", name="xc")
                  for _ in range(4)]
            q4 = [xp.tile([P, 4 * S], BF16, tag="xc", name="xc")
                  for _ in range(4)]
            v4 = [xp.tile([P, 4 * S], BF16, tag="xc", name="xc")
                  for _ in range(4)]

            # ---- PE warmup while first weights stream in ----
            for i in range(NWARM):
                wps = psS.tile([P, P], F32, tag="psS", name="warm")
                nc.tensor.matmul(wps[:], tri_sb[:], tri_sb[:],
                                 start=True, stop=True)

            # ---- input DMA stream: wq, Q, wk, K, wv, V, wo ----
            nc.sync.dma_start(wq_sb[:], wqp[:])
            for c in range(4):
                nc.sync.dma_start(q4[c][:], qtp[:, c * 4 * S:(c + 1) * 4 * S])
            nc.sync.dma_start(wk_sb[:], wkp[:])
            for c in range(4):
                nc.sync.dma_start(k4[c][:], ktp[:, c * 4 * S:(c + 1) * 4 * S])
            nc.sync.dma_start(wv_sb[:], wvp[:])
            for c in range(4):
                nc.sync.dma_start(v4[c][:], vtp[:, c * 4 * S:(c + 1) * 4 * S])
            nc.sync.dma_start(wo_sb[:], wop[:])

            def xsl(tiles, d, a, b):
                return tiles[d // 4][:, (d % 4) * S + a:(d % 4) * S + b]

            # ---- projections (emitted piecewise; see main flow) ----
            def qproj(blk):
                for seg in range(2):
                    a, b2 = seg * SEG, (seg + 1) * SEG
                    pq = psS.tile([P, SEG], F32, tag="psS", name=f"qp{blk}_{seg}")
                    for d in range(DC):
                        nc.tensor.matmul(
                            pq[:],
                            wq_sb[:, d * CQ + blk * P:d * CQ + (blk + 1) * P],
                            xsl(q4, d, a, b2),
                            start=(d == 0), stop=(d == DC - 1),
                        )
                    nc.vector.tensor_copy(qt2[:, blk, a:b2], pq[:])

            def kproj():
                for seg in range(2):
                    a, b2 = seg * SEG, (seg + 1) * SEG
                    ps = psS.tile([P, SEG], F32, tag="psS", name=f"kp{seg}")
                    for d in range(DC):
                        nc.tensor.matmul(
                            ps[:], wk_sb[:, d * CK:(d + 1) * CK],
                            xsl(k4, d, a, b2),
                            start=(d == 0), stop=(d == DC - 1),
                        )
                    nc.scalar.copy(kt2[:, a:b2], ps[:])

            # ---- scores + exp for one pairset (no av) ----
            # Causal mask folded into the PSUM->SBUF evac as an additive -100
            # on the diagonal block, then exp in-place on ACT.
            def score_exp(psx):
                prs = {m: [] for m in range(SC)}
                for m in range(SC):
                    m0 = m * P
                    regions = [(m0, SEG), (SEG, S)] if m0 < SEG else [(m0, S)]
                    for x, base in ((0, 0), (1, HD)):
                        kb = kt2[base:base + HD, m0:m0 + P]
                        for (a, b2) in regions:
                            w = b2 - a
                            sc_ps = psS.tile([P, SEG], F32, tag="psS",
                                             name=f"sc{psx}_{x}_{m}_{a}")
                            nc.tensor.matmul(
                                sc_ps[:, 0:w], kb,
                                qt2[base:base + HD, psx, a:b2],
                                start=True, stop=True,
                            )
                            prx = sxp.tile([P, SEG], BF16, tag="sx",
                                           name=f"pr{psx}_{x}_{m}_{a}")
                            if a == m0:
                                nc.vector.tensor_add(
                                    prx[:, 0:P], sc_ps[:, 0:P], mneg_sb[:]
                                )
                                if w > P:
                                    nc.vector.tensor_copy(
                                        prx[:, P:w], sc_ps[:, P:w]
                                    )
                            else:
                                nc.vector.tensor_copy(prx[:, 0:w], sc_ps[:, 0:w])
                            nc.scalar.activation(
                                prx[:, 0:w], prx[:, 0:w], AF.Exp,
                                scale=1.0 / np.sqrt(HD),
                            )
                            prs[m].append((x, a, b2, prx))
                return prs

            def alloc_oa(psx):
                return [[psO.tile([HD + 1, SEG], F32, tag="psO",
                                  name=f"oa{psx}_{x}_{h}")
                         for h in range(2)] for x in range(2)]

            def av_block(psx, oa, prs, m):
                for (x, a, b2, prx) in prs[m]:
                    half = 0 if a < SEG else 1
                    hb = half * SEG
                    nc.tensor.matmul(
                        oa[x][half][:, a - hb:b2 - hb],
                        vaug[:, m, x, :], prx[:, 0:b2 - a],
                        start=(m == 0),
                        stop=(m == 3 if half == 0 else m == SC - 1),
                    )

            def normalize(psx, half, oa_pair):
                # denominators -> reciprocals -> broadcast across partitions
                # via two concurrent rank-1 col-tiled matmuls (DVE rejects
                # partition-stride-0 APs)
                cs = slice(half * SEG, (half + 1) * SEG)
                recs = []
                for x in range(2):
                    den = npool.tile([1, SEG], F32, tag="den")
                    nc.vector.tensor_copy(den[:], oa_pair[x][HD:HD + 1, :])
                    rec = npool.tile([1, SEG], F32, tag="rec")
                    nc.vector.reciprocal_approx_fast(rec[:], den[:])
                    recs.append(rec)
                rb_ps = psS.tile([P, SEG], F32, tag="psS",
                                 name=f"rb{psx}_{half}")
                nc.tensor.matmul(rb_ps[0:HD, :], ones64f[:], recs[0][:],
                                 start=True, stop=True)
                nc.tensor.matmul(rb_ps[HD:P, :], ones64f[:], recs[1][:],
                                 start=True, stop=True, skip_group_check=True)
                rb_sb = npool.tile([P, SEG], F32, tag="rbc", bufs=1)
                nc.vector.tensor_copy(rb_sb[:], rb_ps[:])
                for x, base in ((0, 0), (1, HD)):
                    nc.vector.tensor_mul(
                        attn_sb[base:base + HD, psx, cs],
                        oa_pair[x][0:HD, :],
                        rb_sb[base:base + HD, :],
                    )

            # ---- collectives + af prefetch ----
            agin = [dp.tile([P, S], BF16, name=f"agin{w}") for w in range(NPS)]
            agout = [dp.tile([4 * P, S], BF16, name=f"agout{w}")
                     for w in range(NPS)]
            af_tiles = [None] * NPS

            def fire_ag(psx):
                nc.sync.dma_start(agin[psx][:], attn_sb[:, psx, :])
                nc.gpsimd.collective_compute(
                    "AllGather",
                    mybir.AluOpType.bypass,
                    replica_groups=[[0, 1, 2, 3], [4, 5, 6, 7]],
                    ins=[agin[psx].opt()],
                    outs=[agout[psx].opt()],
                )
                af = []
                for r in range(4):
                    t = afp.tile([P, S], BF16, tag="af", name=f"af{psx}_{r}")
                    nc.gpsimd.dma_start(t[:], agout[psx][r * P:(r + 1) * P, :])
                    af.append(t)
                af_tiles[psx] = af

            waves_done = [0] * SC

            def oproj_wave(psx):
                af = af_tiles[psx]
                for sc in range(SC):
                    ss = slice(sc * P, (sc + 1) * P)
                    po = psS.tile([P, CO], F32, tag="psS", name=f"po{psx}_{sc}")
                    first = waves_done[sc] == 0
                    if first:
                        nc.tensor.matmul(
                            po[:], ones_sb[:], bo_sb[:], start=True, stop=False,
                        )
                    for r in range(4):
                        c = r * NPS + psx
                        nc.tensor.matmul(
                            po[:], af[r][:, ss], wo_sb[:, c * CO:(c + 1) * CO],
                            start=(r == 0 and not first),
                            stop=(r == 3),
                        )
                    if first:
                        nc.vector.tensor_copy(osum[sc][:], po[:])
                    elif waves_done[sc] == NPS - 1:
                        ot = op.tile([P, CO], F32, tag="osb")
                        nc.vector.tensor_add(ot[:], po[:], osum[sc][:])
                        nc.sync.dma_start(out[sc * P:(sc + 1) * P, :], ot[:])
                    else:
                        nc.vector.tensor_add(osum[sc][:], po[:], osum[sc][:])
                    waves_done[sc] += 1

            # ---- main flow ----
            qproj(0)
            kproj()
            prs0 = score_exp(0)
            qproj(1)
            prs1 = score_exp(1)
            qproj(2)
            qproj(3)

            # v projection: 8 concurrent PSUM chains (m0-3 in psO, which has
            # seen no allocs yet; m4-7 in psS), accumulated in V-chunk-arrival
            # waves so the last matmul lands right behind the last V byte
            pvs = []
            for m in range(SC):
                if m < 4:
                    pvs.append(psO.tile([P, SEG], F32, tag="psO",
                                        name=f"vp{m}"))
                else:
                    pvs.append(psS.tile([P, SEG], F32, tag="psS",
                                        name=f"vp{m}"))
            for c in range(4):
                for m in range(SC):
                    for dd in range(4):
                        d = c * 4 + dd
                        nc.tensor.matmul(
                            pvs[m][:, 0:P], xsl(v4, d, m * P, (m + 1) * P),
                            wv_sb[:, d * CK:(d + 1) * CK],
                            start=(d == 0), stop=(d == DC - 1),
                        )
            for m in range(SC):
                for gl in range(GPC):
                    nc.vector.tensor_copy(
                        vaug[:, m, gl, 0:HD],
                        pvs[m][:, gl * HD:(gl + 1) * HD],
                    )

            oa0 = alloc_oa(0)
            for m in range(SC):
                av_block(0, oa0, prs0, m)
                if m == 3:
                    normalize(0, 0, (oa0[0][0], oa0[1][0]))
            normalize(0, 1, (oa0[0][1], oa0[1][1]))
            fire_ag(0)

            prs2 = score_exp(2)

            oa1 = alloc_oa(1)
            for m in range(SC):
                av_block(1, oa1, prs1, m)
                if m == 3:
                    normalize(1, 0, (oa1[0][0], oa1[1][0]))
            normalize(1, 1, (oa1[0][1], oa1[1][1]))
            fire_ag(1)

            prs3 = score_exp(3)

            oa2 = alloc_oa(2)
            for m in range(SC):
                av_block(2, oa2, prs2, m)
                if m == 3:
                    normalize(2, 0, (oa2[0][0], oa2[1][0]))
            normalize(2, 1, (oa2[0][1], oa2[1][1]))
            fire_ag(2)

            oproj_wave(0)

            oa3 = alloc_oa(3)
            for m in range(SC):
                av_block(3, oa3, prs3, m)
                if m == 3:
                    normalize(3, 0, (oa3[0][0], oa3[1][0]))
            normalize(3, 1, (oa3[0][1], oa3[1][1]))
            fire_ag(3)

            oproj_wave(1)
            oproj_wave(2)
            oproj_wave(3)

    nc.compile()
    return nc


_nc_cache = None


def _pack16(a):
    # [16*128, N] -> [128, 16*N]: row d*128+p -> partition p, chunk d
    n = a.shape[1]
    return np.ascontiguousarray(
        a.reshape(DC, P, n).transpose(1, 0, 2).reshape(P, DC * n)
    )


def build_in_maps(inputs):
    Q = np.asarray(inputs["Q"], np.float32)
    K = np.asarray(inputs["K"], np.float32)
    V = np.asarray(inputs["V"], np.float32)
    w_q = np.asarray(inputs["w_q"], np.float32)
    w_k = np.asarray(inputs["w_k"], np.float32)
    w_v = np.asarray(inputs["w_v"], np.float32)
    w_o = np.asarray(inputs["w_o"], np.float32)
    b_o = np.asarray(inputs["b_o"], np.float32)

    bf = ml_dtypes.bfloat16
    tri = np.triu(np.ones((P, P), np.float32)).astype(bf)
    mneg = np.where(np.triu(np.ones((P, P))) > 0, 0.0, -100.0).astype(
        np.float32
    )

    # w_o contraction rows in chunk order: chunk (r, i) = core r's pairset i
    # = heads (8r+i, 8r+4+i)
    perm = []
    for r in range(4):
        for i in range(NPS):
            perm.extend(range(HD * (8 * r + i), HD * (8 * r + i) + HD))
            perm.extend(range(HD * (8 * r + 4 + i), HD * (8 * r + 4 + i) + HD))
    perm = np.array(perm)

    in_maps = []
    for c in range(NCORES):
        b, j = divmod(c, 4)
        qcols = []
        for i in range(NPS):
            qcols.extend(range(HD * (8 * j + i), HD * (8 * j + i) + HD))
            qcols.extend(range(HD * (8 * j + 4 + i), HD * (8 * j + 4 + i) + HD))
        qcols = np.array(qcols)
        in_maps.append({
            "qtp": _pack16(np.ascontiguousarray(Q[b].T)).astype(bf),
            "ktp": _pack16(np.ascontiguousarray(K[b].T)).astype(bf),
            "vtp": _pack16(np.ascontiguousarray(V[b].T)).astype(bf),
            "wqp": _pack16(np.ascontiguousarray(w_q[qcols, :].T)).astype(bf),
            "wkp": _pack16(
                np.ascontiguousarray(w_k[CK * j:CK * (j + 1), :].T)
            ).astype(bf),
            "wvp": _pack16(
                np.ascontiguousarray(w_v[CK * j:CK * (j + 1), :].T)
            ).astype(bf),
            "wop": _pack16(
                np.ascontiguousarray(w_o[CO * j:CO * (j + 1), :].T[perm, :])
            ).astype(bf),
            "bo": b_o[None, CO * j:CO * (j + 1)].astype(bf),
            "tri": tri,
            "mneg": mneg,
        })
    return in_maps


def kernel(**inputs):
    global _nc_cache
    in_maps = build_in_maps(inputs)
    if _nc_cache is None:
        _nc_cache = _build_nc()
    nc = _nc_cache

    trace = bool(int(os.environ.get("BASS_KERNEL_TRACE", "0")))
    res = bass_utils.run_bass_kernel_spmd(
        nc, in_maps, core_ids=list(range(NCORES)), trace=trace,
    )
    kernel.last_results = res

    out = np.empty((B, S, D), np.float32)
    for c in range(NCORES):
        b, j = divmod(c, 4)
        out[b][:, j * CO:(j + 1) * CO] = res.results[c]["out"]
    return out
